# revision 11
# baseline (speedup 1.0000x reference)
"""TRN2 Bass/Tile kernel: GQA attention with RoPE + logits softcap, causal.

Problem shapes: B=2, T=S=2048, D=2048, N=16 q-heads, K=8 kv-heads, H=128.

Sharding (8 NeuronCores): one KV head (and its G=2 query heads) per core.
Each core computes its heads' full attention and their contribution to the
output projection; the host sums the 8 partial [B,T,D] outputs.

Device algorithm per core (all matmuls in float32r = full-rate fp32):
  - Projections use host-pretransposed x^T [B,D,T]: q^T,k^T,v^T land in
    [H, T] layout (H on partitions). RoPE applied in that layout with
    host-built [128,T] cos/sin tables (partition swap done via DMA).
    v^T is PE-transposed into v [T,H] tiles for the PV matmul.
  - Attention is computed transposed: logits^T [s, tq] tiles, so no
    probability-matrix transposes are ever needed.  Softcap bounds logits
    to +-50 so softmax needs NO max subtraction: p = exp(50*tanh(z*c)).
  - Causal masking: constant wedge bias tiles added to the diagonal-band
    logits before tanh (tanh saturates to -1 -> weight e^-100 relative).
  - Softmax denominators via ones-vector matmul (partition-dim reduction
    on the PE), broadcast back over partitions with a DMA, fused into the
    PSUM->SBUF eviction of enc^T.
  - Output projection: out[tq,d] = sum_g enc^T[g].T @ w_out[g] -- enc^T is
    already in the right layout (H on partitions).
"""

import os
import sys

import numpy as np

for _p in ("/opt/trn_rl_repo", "/root/.axon_site/_ro/trn_rl_repo"):
    if os.path.isdir(_p) and _p not in sys.path:
        sys.path.append(_p)

import concourse.bass as bass
import concourse.mybir as mybir
import concourse.tile as tile
from concourse.bass_utils import run_bass_kernel_spmd
from concourse.masks import make_identity

B, T, D = 2, 2048, 2048
N, KV, H = 16, 8, 128
G = N // KV
SOFTCAP = 50.0
ROPE_BASE = 10000.0
F32 = mybir.dt.float32
R32 = mybir.dt.float32r
C1 = float(1.0 / (SOFTCAP * np.sqrt(H)))  # tanh input scale (folds H^-0.5)
NEG = -30000.0
NCORES = 8
TCH = 512
NTCH = T // TCH  # 4
NDCH = D // 128  # 16
NSB = T // 128  # 16
AF = mybir.ActivationFunctionType


def _r(ap):
    return ap.bitcast(R32)


def legalize_waits(nc, max_waits=1):
    """Split >max_waits semaphore waits onto injected NoOps.

    This walrus codegen only encodes one sync-wait command per engine
    instruction; Tile can emit several, so hoist the excess onto NoOps
    placed just before the instruction on the same engine queue.
    """
    n_added = 0
    for f in nc.m.functions:
        for blk in f.blocks:
            new_insts = []
            changed = False
            for inst in blk.instructions:
                si = inst.sync_info
                waits = list(si.on_wait) if si is not None and si.on_wait else []
                if len(waits) > max_waits:
                    extra, keep = waits[:-max_waits], waits[-max_waits:]
                    for k, w in enumerate(extra):
                        nop = mybir.InstNoOp(
                            name=f"{inst.name}-hw{k}", engine=inst.engine,
                            ins=[], outs=[],
                            sync_info=mybir.SyncInfo(on_wait=[w], on_update=[]),
                        )
                        new_insts.append(nop)
                        n_added += 1
                    inst.sync_info = mybir.SyncInfo(
                        on_wait=keep,
                        on_update=list(si.on_update) if si.on_update else [],
                    )
                    changed = True
                new_insts.append(inst)
            if changed:
                blk.instructions = new_insts
    return n_added


def build_nc():
    nc = bass.Bass()
    xT_d = nc.declare_dram_parameter("xT", [B, D, T], F32, isOutput=False)
    wq_d = nc.declare_dram_parameter("wq", [G, D, H], F32, isOutput=False)
    wk_d = nc.declare_dram_parameter("wk", [D, H], F32, isOutput=False)
    wv_d = nc.declare_dram_parameter("wv", [D, H], F32, isOutput=False)
    wo_d = nc.declare_dram_parameter("wo", [G, H, D], F32, isOutput=False)
    rc_d = nc.declare_dram_parameter("ropec", [B, 128, T], F32, isOutput=False)
    rs_d = nc.declare_dram_parameter("ropes", [B, 128, T], F32, isOutput=False)
    wedge_d = nc.declare_dram_parameter("wedge", [128, 384], F32, isOutput=False)
    out_d = nc.declare_dram_parameter("out", [B, T, D], F32, isOutput=True)

    with tile.TileContext(nc) as tc:
        with (
            tc.tile_pool(name="const", bufs=1) as const,
            tc.tile_pool(name="perb", bufs=1) as perb,
            tc.tile_pool(name="work", bufs=3) as work,
            tc.tile_pool(name="ps1", bufs=1, space="PSUM") as ps1,
            tc.tile_pool(name="ps2", bufs=2, space="PSUM") as ps2,
        ):
            # ---- constants resident in SBUF ----
            wq_sb = const.tile([128, G, NDCH, H], R32, tag="wq")
            for g in range(G):
                nc.sync.dma_start(
                    out=wq_sb[:, g], in_=wq_d[g].bitcast(R32).rearrange("(c p) h -> p c h", p=128)
                )
            wk_sb = const.tile([128, NDCH, H], R32, tag="wk")
            nc.sync.dma_start(out=wk_sb, in_=wk_d[:].bitcast(R32).rearrange("(c p) h -> p c h", p=128))
            wv_sb = const.tile([128, NDCH, H], R32, tag="wv")
            nc.sync.dma_start(out=wv_sb, in_=wv_d[:].bitcast(R32).rearrange("(c p) h -> p c h", p=128))
            wo_sb = const.tile([128, G, D], R32, tag="wo")
            for g in range(G):
                nc.sync.dma_start(out=wo_sb[:, g], in_=wo_d[g].bitcast(R32))
            rc_sb = const.tile([128, B, T], F32, tag="ropec")
            rs_sb = const.tile([128, B, T], F32, tag="ropes")
            for b in range(B):
                nc.sync.dma_start(out=rc_sb[:, b], in_=rc_d[b])
                nc.sync.dma_start(out=rs_sb[:, b], in_=rs_d[b])
            wedge_sb = const.tile([128, 384], F32, tag="wedge")
            nc.sync.dma_start(out=wedge_sb, in_=wedge_d[:])
            ones_f = const.tile([128, 128], F32, tag="ones_f")
            nc.vector.memset(ones_f, 1.0)
            ones_sb = const.tile([128, 1], R32, tag="ones")
            nc.vector.tensor_copy(out=ones_sb, in_=ones_f[:, 0:1])
            onesrow_sb = const.tile([1, 128], R32, tag="onesrow")
            nc.vector.tensor_copy(out=onesrow_sb, in_=ones_f[0:1, :])
            ident_sb = const.tile([128, 128], F32, tag="ident")
            make_identity(nc, ident_sb)

            for b in range(B):
                qT = perb.tile([128, G, T], R32, tag="qT")
                kT = perb.tile([128, T], R32, tag="kT")
                v_sb = perb.tile([128, NSB, H], R32, tag="v")
                encT = perb.tile([128, G, T], R32, tag="encT")

                # ---------- projections (+ rope, + v transpose) ----------
                for tj in range(NTCH):
                    sl = slice(tj * TCH, (tj + 1) * TCH)
                    q0ps = ps1.tile([128, TCH], F32, tag="q0")
                    q1ps = ps1.tile([128, TCH], F32, tag="q1")
                    kps = ps1.tile([128, TCH], F32, tag="k")
                    vps = ps1.tile([128, TCH], F32, tag="vo")
                    for d in range(NDCH):
                        xt = work.tile([128, TCH], R32, tag="xt", bufs=6)
                        nc.sync.dma_start(
                            out=xt, in_=xT_d[b, d * 128 : (d + 1) * 128, sl].bitcast(R32)
                        )
                        st, sp = (d == 0), (d == NDCH - 1)
                        nc.tensor.matmul(q0ps, _r(wq_sb[:, 0, d]), _r(xt), start=st, stop=sp)
                        nc.tensor.matmul(q1ps, _r(wq_sb[:, 1, d]), _r(xt), start=st, stop=sp)
                        nc.tensor.matmul(kps, _r(wk_sb[:, d]), _r(xt), start=st, stop=sp)
                        nc.tensor.matmul(vps, _r(wv_sb[:, d]), _r(xt), start=st, stop=sp)
                    # rope for q0, q1, k (vT has no rope)
                    for ps, dst in ((q0ps, qT[:, 0, sl]), (q1ps, qT[:, 1, sl]), (kps, kT[:, sl])):
                        raw = work.tile([128, TCH], F32, tag="raw")
                        nc.any.tensor_copy(out=raw, in_=ps)
                        rot = work.tile([128, TCH], F32, tag="rot")
                        nc.sync.dma_start(out=rot[0:64], in_=raw[64:128])
                        nc.sync.dma_start(out=rot[64:128], in_=raw[0:64])
                        nc.vector.tensor_mul(out=dst, in0=raw, in1=rc_sb[:, b, sl])
                        nc.vector.tensor_mul(out=rot, in0=rot, in1=rs_sb[:, b, sl])
                        nc.vector.tensor_add(out=dst, in0=dst, in1=rot)
                    # vT psum -> sbuf, then PE-transpose each 128 block into v_sb
                    vT_sb = work.tile([128, TCH], F32, tag="vT")
                    nc.any.tensor_copy(out=vT_sb, in_=vps)
                    for tt in range(4):
                        vtr = ps1.tile([128, 128], F32, tag="sums", name="vtr")
                        nc.tensor.transpose(
                            vtr, vT_sb[:, tt * 128 : (tt + 1) * 128], ident_sb
                        )
                        nc.any.tensor_copy(out=v_sb[:, tj * 4 + tt, :], in_=vtr)

                # ---------- attention (transposed flow) ----------
                for g in range(G):
                    for j in range(NTCH):
                        encps = ps1.tile([128, TCH], F32, tag="enc")
                        sums = ps1.tile([1, TCH], F32, tag="sums")
                        nblk = 4 * j + 4
                        for i in range(nblk):
                            r = i - 4 * j
                            if r < 0:
                                col0, w = 0, TCH
                            elif r < 3:
                                col0, w = 128 * r, TCH - 128 * r
                            else:
                                col0, w = 256, 256
                            lg_full = ps2.tile([128, TCH], F32, tag="lg", name="lg")
                            lg = lg_full[:, :w]
                            nc.tensor.matmul(
                                lg,
                                _r(kT[:, i * 128 : (i + 1) * 128]),
                                _r(qT[:, g, j * TCH + col0 : (j + 1) * TCH]),
                                start=True,
                                stop=True,
                            )
                            if r >= 0:
                                if r < 3:
                                    nc.vector.tensor_add(
                                        out=lg[:, 0:128], in0=lg[:, 0:128],
                                        in1=wedge_sb[:, 0:128],
                                    )
                                else:
                                    nc.vector.tensor_add(
                                        out=lg[:, 0:256], in0=lg[:, 0:256],
                                        in1=wedge_sb[:, 128:384],
                                    )
                            pt = work.tile([128, TCH], R32, tag="pt", bufs=4)
                            nc.scalar.activation(out=pt[:, :w], in_=lg, func=AF.Tanh, scale=C1)
                            nc.scalar.activation(out=pt[:, :w], in_=pt[:, :w], func=AF.Exp, scale=SOFTCAP)
                            st, sp = (i == 0), (i == nblk - 1)
                            nc.tensor.matmul(
                                sums[:, col0:TCH], _r(ones_sb), _r(pt[:, :w]),
                                start=st, stop=sp,
                            )
                            nc.tensor.matmul(
                                encps[:, col0:TCH], _r(v_sb[:, i, :]), _r(pt[:, :w]),
                                start=st, stop=sp,
                            )
                        rcp = work.tile([1, TCH], R32, tag="rcp", bufs=2)
                        with nc.allow_low_precision(reason="fp32r rounding of softmax denominators"):
                            nc.vector.reciprocal(out=rcp, in_=sums)
                        bcps = ps2.tile([128, TCH], F32, tag="lg", name="bcps")
                        nc.tensor.matmul(bcps, onesrow_sb, rcp, start=True, stop=True)
                        bc = work.tile([128, TCH], F32, tag="bc", bufs=2)
                        nc.any.tensor_copy(out=bc, in_=bcps)
                        nc.vector.tensor_mul(
                            out=encT[:, g, j * TCH : (j + 1) * TCH], in0=encps, in1=bc
                        )

                # ---------- output projection ----------
                for ti in range(NSB):
                    for dj in range(NTCH):
                        ops = ps2.tile([128, TCH], F32, tag="lg")
                        for g in range(G):
                            nc.tensor.matmul(
                                ops,
                                _r(encT[:, g, ti * 128 : (ti + 1) * 128]),
                                _r(wo_sb[:, g, dj * TCH : (dj + 1) * TCH]),
                                start=(g == 0),
                                stop=(g == G - 1),
                            )
                        ob = work.tile([128, TCH], F32, tag="ob")
                        nc.any.tensor_copy(out=ob, in_=ops)
                        nc.sync.dma_start(
                            out=out_d[b, ti * 128 : (ti + 1) * 128, dj * TCH : (dj + 1) * TCH],
                            in_=ob,
                        )
    legalize_waits(nc)
    return nc


_CACHE = {}


def _host_prep(x, positions, attn_mask, w_q, w_kv, w_out):
    x = np.ascontiguousarray(np.asarray(x), dtype=np.float32)
    positions = np.asarray(positions)
    attn_mask = np.asarray(attn_mask)
    w_q = np.asarray(w_q, dtype=np.float32)
    w_kv = np.asarray(w_kv, dtype=np.float32)
    w_out = np.asarray(w_out, dtype=np.float32)

    tril = np.tril(np.ones((T, T), dtype=bool))
    for b in range(B):
        if not np.array_equal(attn_mask[b, 0], tril):
            raise NotImplementedError("only the causal mask is supported")

    pos = positions.astype(np.float32)  # [B,T]
    frac = 2.0 * np.arange(H // 2, dtype=np.float32) / H
    timescale = (ROPE_BASE**frac).astype(np.float32)  # [64]
    ang = pos[:, None, :] / timescale[None, :, None]  # [B,64,T]
    cos = np.cos(ang).astype(np.float32)
    sin = np.sin(ang).astype(np.float32)
    ropec = np.ascontiguousarray(np.concatenate([cos, cos], axis=1))  # [B,128,T]
    ropes = np.ascontiguousarray(np.concatenate([-sin, sin], axis=1))

    p_idx = np.arange(128, dtype=np.int64)[:, None]
    w0 = np.where(np.arange(128)[None, :] >= p_idx, 0.0, NEG).astype(np.float32)
    w1 = np.where(np.arange(256)[None, :] >= 128 + p_idx, 0.0, NEG).astype(np.float32)
    wedge = np.ascontiguousarray(np.concatenate([w0, w1], axis=1))  # [128,384]

    xT = np.ascontiguousarray(x.transpose(0, 2, 1))  # [B,D,T]

    in_maps = []
    for c in range(NCORES):
        in_maps.append(
            {
                "xT": xT,
                "wq": np.ascontiguousarray(w_q[c * G : (c + 1) * G]),
                "wk": np.ascontiguousarray(w_kv[0, c]),
                "wv": np.ascontiguousarray(w_kv[1, c]),
                "wo": np.ascontiguousarray(w_out[c * G : (c + 1) * G]),
                "ropec": ropec,
                "ropes": ropes,
                "wedge": wedge,
            }
        )
    return in_maps


def run(in_maps, **kw):
    if "nc" not in _CACHE:
        _CACHE["nc"] = build_nc()
    return run_bass_kernel_spmd(_CACHE["nc"], in_maps, core_ids=list(range(NCORES)), **kw)


def kernel(x, positions, attn_mask, w_q, w_kv, w_out):
    in_maps = _host_prep(x, positions, attn_mask, w_q, w_kv, w_out)
    res = run(in_maps)
    acc = np.zeros((B, T, D), dtype=np.float64)
    for r in res.results:
        acc += r["out"].astype(np.float64)
    return acc.astype(np.float32)


# revision 23
# speedup vs baseline: 12.1348x; 12.1348x over previous
"""TRN2 Bass/Tile kernel: GQA attention with RoPE + logits softcap, causal.

Problem shapes: B=2, T=S=2048, D=2048, N=16 q-heads, K=8 kv-heads, H=128.

Sharding (8 NeuronCores): one KV head (and its G=2 query heads) per core.
Each core computes its heads' full attention and their contribution to the
output projection; the host sums the 8 partial [B,T,D] outputs.

Device algorithm per core (all matmuls in float32r = full-rate fp32 on the
PE when the moving free dim >= 256; ~1.5e-4 matmul relative error):
  - Projections use host-pretransposed x^T [B,D,T]: q^T,k^T,v^T land in
    [H, T] layout (H on partitions, contraction over D in 16 chunks).
    RoPE is applied in that layout with host-built [128,T] cos/sin tables;
    the half-rotation partner (a cross-partition swap) is produced by a
    PE matmul with a constant permutation matrix.  v^T is PE-transposed
    into v [T,H] tiles for the PV matmul.
  - Attention is computed transposed: logits^T [s, tq] tiles
    (lhsT=k^T block, rhs=q^T chunk), so no probability-matrix transposes
    are ever needed.  The softcap bounds logits to +-50 so the softmax
    needs NO max subtraction: p = exp(50*tanh(z*c1)), computed as two
    ScalarE activations, pipelined DEPTH blocks ahead of the PE consumers.
  - Causal masking: constant wedge bias tiles (host input) added to the
    diagonal-band logits before tanh (tanh saturates to -1 exactly, so
    masked entries get relative weight ~e^-100); diagonal-band blocks are
    column-narrowed to skip fully-masked regions.
  - Softmax denominators: matmul with an all-ones [128,128] stationary
    reduces over the partition (s) dim AND broadcasts the sums to all 128
    partitions in one shot; normalization is a reciprocal + multiply fused
    into the PSUM->SBUF eviction of enc^T.
  - Output projection: out[tq,d] = sum_g enc^T[g].T @ w_out[g] -- enc^T is
    already in the right layout (H on partitions).
  - Schedule: one software pipeline per batch interleaving projection
    chunk j, attention chunks (g, j) (which only need K/V up to chunk j,
    by causality), and the previous chunk's output-projection iterations
    as PE filler inside the ACT-bound attention block loops.  The
    single-wait legalization pass (legalize_waits) adapts Tile output to
    this walrus codegen.
"""

import os
import sys

import numpy as np

for _p in ("/opt/trn_rl_repo", "/root/.axon_site/_ro/trn_rl_repo"):
    if os.path.isdir(_p) and _p not in sys.path:
        sys.path.append(_p)

import concourse.bass as bass
import concourse.mybir as mybir
import concourse.tile as tile
from concourse.bass_utils import run_bass_kernel_spmd
from concourse.masks import make_identity

B, T, D = 2, 2048, 2048
N, KV, H = 16, 8, 128
G = N // KV
SOFTCAP = 50.0
ROPE_BASE = 10000.0
F32 = mybir.dt.float32
R32 = mybir.dt.float32r
C1 = float(1.0 / (SOFTCAP * np.sqrt(H)))  # tanh input scale (folds H^-0.5)
NEG = -30000.0
NCORES = 8
TCH = 512
NTCH = T // TCH  # 4
NDCH = D // 128  # 16
NSB = T // 128  # 16
AF = mybir.ActivationFunctionType


def legalize_waits(nc, max_waits=1):
    """Split >max_waits semaphore waits onto injected NoOps.

    This walrus codegen only encodes one sync-wait command per engine
    instruction; Tile can emit several, so hoist the excess onto NoOps
    placed just before the instruction on the same engine queue.
    """
    n_added = 0
    for f in nc.m.functions:
        for blk in f.blocks:
            new_insts = []
            changed = False
            for inst in blk.instructions:
                si = inst.sync_info
                waits = list(si.on_wait) if si is not None and si.on_wait else []
                if len(waits) > max_waits:
                    extra, keep = waits[:-max_waits], waits[-max_waits:]
                    for k, w in enumerate(extra):
                        nop = mybir.InstNoOp(
                            name=f"{inst.name}-hw{k}", engine=inst.engine,
                            ins=[], outs=[],
                            sync_info=mybir.SyncInfo(on_wait=[w], on_update=[]),
                        )
                        new_insts.append(nop)
                        n_added += 1
                    inst.sync_info = mybir.SyncInfo(
                        on_wait=keep,
                        on_update=list(si.on_update) if si.on_update else [],
                    )
                    changed = True
                new_insts.append(inst)
            if changed:
                blk.instructions = new_insts
    return n_added


def build_nc(repeat=1):
    nc = bass.Bass()
    xT_d = nc.declare_dram_parameter("xT", [B, D, T], F32, isOutput=False)
    wq_d = nc.declare_dram_parameter("wq", [G, D, H], F32, isOutput=False)
    wk_d = nc.declare_dram_parameter("wk", [D, H], F32, isOutput=False)
    wv_d = nc.declare_dram_parameter("wv", [D, H], F32, isOutput=False)
    wo_d = nc.declare_dram_parameter("wo", [G, H, D], F32, isOutput=False)
    rc_d = nc.declare_dram_parameter("ropec", [B, 128, T], F32, isOutput=False)
    rs_d = nc.declare_dram_parameter("ropes", [B, 128, T], F32, isOutput=False)
    wedge_d = nc.declare_dram_parameter("wedge", [128, 384], F32, isOutput=False)
    perm_d = nc.declare_dram_parameter("perm", [128, 128], F32, isOutput=False)
    out_d = nc.declare_dram_parameter("out", [B, T, D], F32, isOutput=True)

    with tile.TileContext(nc) as tc:
        with (
            tc.tile_pool(name="const", bufs=1) as const,
            tc.tile_pool(name="perb", bufs=1) as perb,
            tc.tile_pool(name="work", bufs=3) as work,
            tc.tile_pool(name="ps1", bufs=1, space="PSUM") as ps1,
            tc.tile_pool(name="ps2", bufs=2, space="PSUM") as ps2,
        ):
            # ---- constants resident in SBUF (big late-needed ones deferred) ----
            wq_sb = const.tile([128, G, NDCH, H], R32, tag="wq")
            wk_sb = const.tile([128, NDCH, H], R32, tag="wk")
            wv_sb = const.tile([128, NDCH, H], R32, tag="wv")
            nc.sync.dma_start(
                out=wq_sb[:, 0], in_=wq_d[0].bitcast(R32).rearrange("(c p) h -> p c h", p=128)
            )
            nc.scalar.dma_start(
                out=wq_sb[:, 1], in_=wq_d[1].bitcast(R32).rearrange("(c p) h -> p c h", p=128)
            )
            nc.sync.dma_start(out=wk_sb, in_=wk_d[:].bitcast(R32).rearrange("(c p) h -> p c h", p=128))
            nc.scalar.dma_start(out=wv_sb, in_=wv_d[:].bitcast(R32).rearrange("(c p) h -> p c h", p=128))
            wedge_sb = const.tile([128, 384], F32, tag="wedge")
            nc.scalar.dma_start(out=wedge_sb, in_=wedge_d[:])
            ones_f = const.tile([128, 128], F32, tag="ones_f")
            nc.vector.memset(ones_f, 1.0)
            ones128_sb = const.tile([128, 128], R32, tag="ones")
            nc.vector.tensor_copy(out=ones128_sb, in_=ones_f)
            ident_sb = const.tile([128, 128], F32, tag="ident")
            make_identity(nc, ident_sb)
            perm_sb = const.tile([128, 128], R32, tag="perm")
            nc.scalar.dma_start(out=perm_sb, in_=perm_d[:].bitcast(R32))
            wo_sb = const.tile([128, G, D], R32, tag="wo")
            rc_sb = const.tile([128, B, T], F32, tag="ropec")
            rs_sb = const.tile([128, B, T], F32, tag="ropes")

            def proj_chunk(b, tj, qT, kT, v_sb):
                sl = slice(tj * TCH, (tj + 1) * TCH)
                q0ps = ps1.tile([128, TCH], F32, tag="q0", name="q0ps")
                q1ps = ps1.tile([128, TCH], F32, tag="q1", name="q1ps")
                kps = ps1.tile([128, TCH], F32, tag="k", name="kps")
                vps = ps1.tile([128, TCH], F32, tag="voenc", name="vps", bufs=2)
                for d in range(NDCH):
                    xt = work.tile([128, TCH], R32, tag="xt", bufs=16, name="xt")
                    dma_eng = nc.sync if d % 2 == 0 else nc.scalar
                    dma_eng.dma_start(
                        out=xt, in_=xT_d[b, d * 128 : (d + 1) * 128, sl].bitcast(R32)
                    )
                    st, sp = (d == 0), (d == NDCH - 1)
                    nc.tensor.matmul(q0ps, wq_sb[:, 0, d], xt, start=st, stop=sp)
                    nc.tensor.matmul(q1ps, wq_sb[:, 1, d], xt, start=st, stop=sp)
                    nc.tensor.matmul(kps, wk_sb[:, d], xt, start=st, stop=sp)
                    nc.tensor.matmul(vps, wv_sb[:, d], xt, start=st, stop=sp)
                if b == 0 and tj == 0:
                    # tables are first needed by the rope just below; the DMAs
                    # queue behind the first x chunk instead of ahead of it
                    nc.scalar.dma_start(out=rc_sb[:, 0], in_=rc_d[0])
                    nc.scalar.dma_start(out=rs_sb[:, 0], in_=rs_d[0])
                # rope for q0, q1, k (v has no rope)
                for ps, dst in ((q0ps, qT[:, 0, sl]), (q1ps, qT[:, 1, sl]), (kps, kT[:, sl])):
                    raw = work.tile([128, TCH], R32, tag="raw", bufs=2, name="raw")
                    nc.scalar.copy(out=raw, in_=ps)
                    rotps = ps2.tile([128, TCH], F32, tag="lg", name="rotps")
                    nc.tensor.matmul(rotps, perm_sb, raw, start=True, stop=True)
                    rot = work.tile([128, TCH], F32, tag="rot", bufs=2, name="rot")
                    nc.vector.tensor_mul(out=rot, in0=rotps, in1=rs_sb[:, b, sl])
                    nc.vector.tensor_mul(out=dst, in0=raw, in1=rc_sb[:, b, sl])
                    nc.vector.tensor_add(out=dst, in0=dst, in1=rot)
                # vT psum -> sbuf, then PE-transpose each 128 block into v_sb
                vT_sb = work.tile([128, TCH], F32, tag="vT", bufs=2, name="vT")
                nc.scalar.copy(out=vT_sb, in_=vps)
                for tt in range(4):
                    vtr = ps1.tile([128, 128], F32, tag="sums", name="vtr")
                    nc.tensor.transpose(vtr, vT_sb[:, tt * 128 : (tt + 1) * 128], ident_sb)
                    nc.vector.tensor_copy(out=v_sb[:, tj * 4 + tt, :], in_=vtr)

            def attn_chunk(b, g, j, qT, kT, v_sb, encT, filler=None):
                encps = ps1.tile([128, TCH], F32, tag="voenc", name="encps", bufs=2)
                sums = ps1.tile([128, TCH], F32, tag="sums", name="sums")
                nblk = 4 * j + 4

                def blk_geom(i):
                    r = i - 4 * j
                    if r < 0:
                        return 0, TCH
                    if r < 3:
                        return 128 * r, TCH - 128 * r
                    return 256, 256

                pts = {}
                DEPTH = 6
                for i in range(nblk + DEPTH):
                    if filler is not None and i % 2 == 0:
                        filler()
                    if i < nblk:
                        col0, w = blk_geom(i)
                        r = i - 4 * j
                        lg_full = ps2.tile([128, TCH], F32, tag="lg", name="lg")
                        lg = lg_full[:, :w]
                        nc.tensor.matmul(
                            lg,
                            kT[:, i * 128 : (i + 1) * 128],
                            qT[:, g, j * TCH + col0 : (j + 1) * TCH],
                            start=True,
                            stop=True,
                        )
                    if i >= DEPTH:
                        # PE consumes probs DEPTH blocks behind ACT
                        ic = i - DEPTH
                        pcol0, pw = blk_geom(ic)
                        ppt = pts.pop(ic)
                        st, sp = (ic == 0), (ic == nblk - 1)
                        nc.tensor.matmul(
                            sums[:, pcol0:TCH], ones128_sb, ppt[:, :pw], start=st, stop=sp
                        )
                        nc.tensor.matmul(
                            encps[:, pcol0:TCH], v_sb[:, ic, :], ppt[:, :pw],
                            start=st, stop=sp,
                        )
                    if i < nblk:
                        if r >= 0:
                            if r < 3:
                                nc.vector.tensor_add(
                                    out=lg[:, 0:128], in0=lg[:, 0:128],
                                    in1=wedge_sb[:, 0:128],
                                )
                            else:
                                nc.vector.tensor_add(
                                    out=lg[:, 0:256], in0=lg[:, 0:256],
                                    in1=wedge_sb[:, 128:384],
                                )
                        pt = work.tile([128, TCH], R32, tag="pt", bufs=8, name="pt")
                        nc.scalar.activation(out=pt[:, :w], in_=lg, func=AF.Tanh, scale=C1)
                        nc.scalar.activation(out=pt[:, :w], in_=pt[:, :w], func=AF.Exp, scale=SOFTCAP)
                        pts[i] = pt
                bc = work.tile([128, TCH], F32, tag="bc", bufs=3, name="bc")
                nc.vector.reciprocal(out=bc, in_=sums)
                nc.vector.tensor_mul(
                    out=encT[:, g, j * TCH : (j + 1) * TCH], in0=encps, in1=bc
                )

            def outproj_iter(b, ti, dj, encT):
                        ops = ps2.tile([128, TCH], F32, tag="lg", name="ops")
                        for g in range(G):
                            nc.tensor.matmul(
                                ops,
                                encT[:, g, ti * 128 : (ti + 1) * 128],
                                wo_sb[:, g, dj * TCH : (dj + 1) * TCH],
                                start=(g == 0),
                                stop=(g == G - 1),
                            )
                        ob = work.tile([128, TCH], F32, tag="ob", bufs=5, name="ob")
                        nc.vector.tensor_copy(out=ob, in_=ops)
                        nc.sync.dma_start(
                            out=out_d[b, ti * 128 : (ti + 1) * 128, dj * TCH : (dj + 1) * TCH],
                            in_=ob,
                        )

            for _rep in range(repeat):
              for b in range(B):
                qT = perb.tile([128, G, T], R32, tag="qT", name="qT")
                kT = perb.tile([128, T], R32, tag="kT", name="kT")
                v_sb = perb.tile([128, NSB, H], R32, tag="v", name="v_sb")
                encT = perb.tile([128, G, T], R32, tag="encT", name="encT")
                def make_filler(jsrc, encT_src):
                    iters = [(ti, dj) for ti in range(4 * jsrc, 4 * jsrc + 4)
                             for dj in range(NTCH)]
                    it = iter(iters)

                    def _f():
                        nxt = next(it, None)
                        if nxt is not None:
                            outproj_iter(b, nxt[0], nxt[1], encT_src)

                    def _drain():
                        for ti, dj in it:
                            outproj_iter(b, ti, dj, encT_src)

                    return _f, _drain

                drain_prev = None
                for j in range(NTCH):
                    proj_chunk(b, j, qT, kT, v_sb)
                    if b == 0 and j == 0:
                        # deferred big constants, needed from the first
                        # attention/outproj chunks onward
                        nc.scalar.dma_start(out=rc_sb[:, 1], in_=rc_d[1])
                        nc.scalar.dma_start(out=rs_sb[:, 1], in_=rs_d[1])
                        for g in range(G):
                            nc.scalar.dma_start(out=wo_sb[:, g], in_=wo_d[g].bitcast(R32))
                    filler = drain_prev[0] if drain_prev is not None else None
                    for g in range(G):
                        attn_chunk(b, g, j, qT, kT, v_sb, encT, filler=filler)
                    if drain_prev is not None:
                        drain_prev[1]()
                    drain_prev = make_filler(j, encT)
                drain_prev[1]()
    legalize_waits(nc)
    return nc


_CACHE = {}


def _host_prep(x, positions, attn_mask, w_q, w_kv, w_out):
    x = np.ascontiguousarray(np.asarray(x), dtype=np.float32)
    positions = np.asarray(positions)
    attn_mask = np.asarray(attn_mask)
    w_q = np.asarray(w_q, dtype=np.float32)
    w_kv = np.asarray(w_kv, dtype=np.float32)
    w_out = np.asarray(w_out, dtype=np.float32)

    tril = np.tril(np.ones((T, T), dtype=bool))
    for b in range(B):
        if not np.array_equal(attn_mask[b, 0], tril):
            raise NotImplementedError("only the causal mask is supported")

    pos = positions.astype(np.float32)  # [B,T]
    frac = 2.0 * np.arange(H // 2, dtype=np.float32) / H
    timescale = (ROPE_BASE**frac).astype(np.float32)  # [64]
    ang = pos[:, None, :] / timescale[None, :, None]  # [B,64,T]
    cos = np.cos(ang).astype(np.float32)
    sin = np.sin(ang).astype(np.float32)
    ropec = np.ascontiguousarray(np.concatenate([cos, cos], axis=1))  # [B,128,T]
    ropes = np.ascontiguousarray(np.concatenate([-sin, sin], axis=1))

    p_idx = np.arange(128, dtype=np.int64)[:, None]
    w0 = np.where(np.arange(128)[None, :] >= p_idx, 0.0, NEG).astype(np.float32)
    w1 = np.where(np.arange(256)[None, :] >= 128 + p_idx, 0.0, NEG).astype(np.float32)
    wedge = np.ascontiguousarray(np.concatenate([w0, w1], axis=1))  # [128,384]

    xT = np.ascontiguousarray(x.transpose(0, 2, 1))  # [B,D,T]
    perm = np.zeros((128, 128), np.float32)
    # rot[p] = raw[(p+64)%128]; out = lhsT.T @ rhs -> lhsT[q, p] = 1 iff q == (p+64)%128
    for p in range(128):
        perm[(p + 64) % 128, p] = 1.0

    in_maps = []
    for c in range(NCORES):
        in_maps.append(
            {
                "xT": xT,
                "wq": np.ascontiguousarray(w_q[c * G : (c + 1) * G]),
                "wk": np.ascontiguousarray(w_kv[0, c]),
                "wv": np.ascontiguousarray(w_kv[1, c]),
                "wo": np.ascontiguousarray(w_out[c * G : (c + 1) * G]),
                "ropec": ropec,
                "ropes": ropes,
                "wedge": wedge,
                "perm": perm,
            }
        )
    return in_maps


def run(in_maps, **kw):
    if "nc" not in _CACHE:
        _CACHE["nc"] = build_nc()
    return run_bass_kernel_spmd(_CACHE["nc"], in_maps, core_ids=list(range(NCORES)), **kw)


def kernel(x, positions, attn_mask, w_q, w_kv, w_out):
    in_maps = _host_prep(x, positions, attn_mask, w_q, w_kv, w_out)
    res = run(in_maps)
    acc = np.zeros((B, T, D), dtype=np.float64)
    for r in res.results:
        acc += r["out"].astype(np.float64)
    return acc.astype(np.float32)

